# revision 1
# baseline (speedup 1.0000x reference)
r"""Trainium2 Bass kernel for the CounterfactualODEModel problem.

Reference computes an adaptive dopri5 solve of
    dx/dt = MLP(concat(x, tr(t))),  tr = piecewise-linear treatments,
evaluated at the T=100 grid times.  Instead of replaying the sequential
solver on device, this kernel uses a parallel-in-time Picard iteration on
the integral form  x(t) = x0 + \int_0^t f(x(s), s) ds:

  - sample times = the 100 grid points plus the 99 interval midpoints
    (S = 199); within one interval the treatments are linear, so composite
    Simpson over each interval gives O(h^4) quadrature accuracy with no
    error from the treatment kinks at grid points,
  - each sweep evaluates the MLP at all S times as batched matmuls and
    applies the cumulative-quadrature matrix A (built on host from ts):
        X <- x0 + A @ f(X),
  - the iteration contracts ~25x per sweep; early sweeps run in fast
    float32r (TF32-like, ~1.5e-4 accuracy), the last sweep(s) in full
    fp32 polish the fixed point back to fp32 accuracy.  The converged
    trajectory is ~9e-8 from the true f64 solution; the f32 dopri5
    reference itself sits ~2e-5 away.

Implementation notes:
  - raw Bass (explicit engine streams + semaphores): the walrus build in
    this environment rejects instructions carrying more than one attached
    sync-wait, which rules out Tile-generated scheduling; standalone
    wait_ge instructions sidestep that limit,
  - f32r matmuls with the moving free dim padded to 256 run at 1
    cycle/row vs fp32's 4 (fp32 lowers to two half-speed passes plus a
    double weight load); constants consumed by 'r' sweeps ride a
    float32r-typed DMA (quantized in transit), the fp32 polish sweeps
    get exact float32 copies,
  - inputs are split into per-region DMAs with exact partition counts so
    they ride parallel HWDGE queues; two semaphore groups let the MLP
    matmuls of sweep 0 start before the big quadrature matrix lands.

The whole state is tiny (S x 36 floats), so the problem is replicated on
all 8 cores (no useful tensor/batch parallelism exists for one
trajectory); core 0's output is returned.
"""

import numpy as np

from contextlib import ExitStack

import concourse.bass as bass
import concourse.mybir as mybir
from concourse import bass_utils

T = 100
S = 2 * T - 1  # grid + midpoints
SP = 256       # padded free dim (f32r matmul runs 1 cycle/row at >=256)
FD = 32   # feature dim
TD = 4    # treatment dim
HD = 64   # hidden dim
IN_DIM = FD + TD
PLAN = "rrrf"     # per-sweep precision per char:
#   r = all float32r   f = all float32
#   g = float32 MLP + float32r integration
#   m = float32r MLP + float32 integration
K1 = 128          # partition-chunk split of the S-long contraction
K2 = S - K1
N_CORES = 8
NDMY = 0       # PE p-state filler matmuls per sem-wait gap (no gain on HW)

_DT = mybir.dt.float32
_R = mybir.dt.float32r

# D1 (f32r, [64, 416]): state | W1 | W2 | W3       -- PE-critical, lands first
_D1_ST = 0
_D1_W1 = _D1_ST + SP
_D1_W2 = _D1_W1 + HD
_D1_W3 = _D1_W2 + HD
D1W = _D1_W3 + FD
# D2 (f32, [64, 258]): b1 | b2 | DM   (small: unblocks ACT/DVE fast)
_D2_B1 = 0
_D2_B2 = _D2_B1 + 1
_D2_DM = _D2_B2 + 1
D2W = _D2_DM + SP
# D5 (f32, [128, 930]): ATA_f | ATB_f | W1f|W2f|W3f | STf  (deferred: only
# needed by the fp32 polish sweep)
_D5_ATA = 0
_D5_ATB = _D5_ATA + SP
_D5_W1 = _D5_ATB + SP
_D5_W2 = _D5_W1 + HD
_D5_W3 = _D5_W2 + HD
_D5_ST = _D5_W3 + FD
D5W = _D5_ST + SP


class _LeanBlock(bass.BassBlock):
    """Block whose exit skips the all-engine EVSEM butterfly: engines just
    drain and end.  Output integrity is guaranteed by the sync stream's
    final wait on the output-DMA semaphore; semaphores are re-cleared by
    the preamble on every execution."""

    def __exit__(self, exc_type, exc_val, exc_tb):
        if exc_type is not None:
            return
        for engine, last_body in self.last_body.items():
            with self.bass.body(
                last_body, parent=self.bass.cur_bb, allow_existing_parent=True
            ):
                engine.br(self.end_bb)
        self.bass.switch_bb(self.end_bb)
        gpsimd_type = self.bass.gpsimd.engine
        for eng_type, eng in self.bass.engines.items():
            if eng_type == gpsimd_type:
                continue
            d = mybir.InstDrain(
                name=self.bass.get_next_instruction_name(),
                ins=[],
                outs=[],
                bass_is_fusable=False,
            )
            d.engine = eng_type
            eng.add_instruction(d)


def _build_nc(plan=PLAN):
    nsweep = len(plan)
    nc = bass.Bass(trn_type="TRN2", monotonic_sem_count=0, enable_partition_id=False)
    d_1 = nc.dram_tensor("d1", [HD, D1W], _R, kind="ExternalInput")
    d_2 = nc.dram_tensor("d2", [HD, D2W], _DT, kind="ExternalInput")
    d_3 = nc.dram_tensor("d3", [K1, SP], _R, kind="ExternalInput")
    d_4 = nc.dram_tensor("d4", [K2, SP], _R, kind="ExternalInput")
    d_5 = nc.dram_tensor("d5", [K1, D5W], _DT, kind="ExternalInput")
    xt = nc.dram_tensor("xt", [FD, S], _DT, kind="ExternalOutput")

    tanh = mybir.ActivationFunctionType.Tanh

    with ExitStack() as ctx:
        sb = lambda nm, shape, dt: ctx.enter_context(nc.sbuf_tensor(nm, shape, dt))
        ps = lambda nm, shape: ctx.enter_context(nc.psum_tensor(nm, shape, _DT))
        sem = lambda nm: ctx.enter_context(nc.semaphore(nm))
        t1 = sb("t_d1", [HD, D1W], _R)
        t2 = sb("t_d2", [HD, D2W], _DT)
        ata_r = sb("t_ata_r", [K1, SP], _R)
        atb_r = sb("t_atb_r", [K2, SP], _R)
        t5 = sb("t_d5", [K1, D5W], _DT)
        h1 = sb("t_h1", [HD, SP], _R)
        h1f = sb("t_h1f", [HD, SP], _DT)
        h2 = sb("t_h2", [HD, SP], _R)
        h2f = sb("t_h2f", [HD, SP], _DT)
        fab = sb("t_fab", [K1, 2 * FD], _R)
        fabf = sb("t_fabf", [K1, 2 * FD], _DT)
        warm = sb("t_warm", [HD, 1], _DT)
        p1 = ps("t_p1", [HD, SP])
        p2 = ps("t_p2", [HD, SP])
        pf = ps("t_pf", [K1, 2 * FD])
        px = ps("t_px", [FD, SP])
        pdmy = ps("t_pdmy", [1, 16])
        sem_w = sem("sem_w")
        sem_b = sem("sem_b")
        sem_a3 = sem("sem_a3")
        sem_a4 = sem("sem_a4")
        sem_a5 = sem("sem_a5")
        pe_sem = sem("sem_pe")
        act_sem = sem("sem_act")
        dve_sem = sem("sem_dve")
        gp_sem = sem("sem_gp")
        stt = t1[0:IN_DIM, _D1_ST:_D1_ST + SP]
        sttf = t5[0:IN_DIM, _D5_ST:_D5_ST + SP]
        block = ctx.enter_context(_LeanBlock(nc, 'blk'))

        rops = {
            "ata": ata_r[:, :], "atb": atb_r[:, :],
            "w1": t1[0:IN_DIM, _D1_W1:_D1_W1 + HD],
            "w2": t1[0:HD, _D1_W2:_D1_W2 + HD],
            "w3": t1[0:HD, _D1_W3:_D1_W3 + FD],
            "h1": h1, "h2": h2, "fab": fab,
        }
        fops = {
            "ata": t5[0:K1, _D5_ATA:_D5_ATA + SP], "atb": t5[0:K2, _D5_ATB:_D5_ATB + SP],
            "w1": t5[0:IN_DIM, _D5_W1:_D5_W1 + HD],
            "w2": t5[0:HD, _D5_W2:_D5_W2 + HD],
            "w3": t5[0:HD, _D5_W3:_D5_W3 + FD],
            "h1": h1f, "h2": h2f, "fab": fabf,
        }
        b1t = t2[0:HD, _D2_B1:_D2_B1 + 1]
        b2t = t2[0:HD, _D2_B2:_D2_B2 + 1]
        dm = t2[0:FD, _D2_DM:_D2_DM + SP]

        # MLP precision: 'r'/'m' run the MLP in f32r (state lives in the
        # f32r stt tile); 'f'/'g' in fp32 (state in sttf inside t5).
        mlp_r = lambda c: c in "rm"
        int_r = lambda c: c in "rg"

        def state_in(j):
            if j < len(plan):
                return stt if mlp_r(plan[j]) else sttf
            return stt if mlp_r(plan[-1]) else sttf

        # semaphore plan (DMA sems inc by 16 at transfer completion):
        #   sem_w:  D1 (state+weights, f32r) = 16; output DMA -> 32
        #   sem_b:  D2 (biases + DM, f32)    = 16
        #   sem_a3/a4: ATA/ATB (f32r)        = 16 each
        #   sem_a5: D5 (all fp32-sweep data) = 16
        #   pe_sem:  6 matmuls/sweep -> 6j+k after k-th matmul of sweep j
        #   act_sem: 2 tanhs/sweep   -> 2j+k
        #   dve_sem: 2 ops/sweep     -> 2j+k
        #   gp_sem:  4 pad-zero memsets

        @block.gpsimd
        def _(gpsimd):
            for t in (h1, h2):
                nc.gpsimd.memset(t.bitcast(_DT)[:, :], 0.0).then_inc(gp_sem, 1)
            # D1 split: state+W1 land first (gates mm1), W2/W3 follow
            nc.gpsimd.dma_start(t1[:, 0:_D1_W2], d_1[:, 0:_D1_W2]).then_inc(sem_w, 16)
            nc.gpsimd.dma_start(t1[:, _D1_W2:D1W], d_1[:, _D1_W2:D1W]).then_inc(sem_w, 16)
            nc.gpsimd.dma_start(atb_r[:, :], d_4[:, :]).then_inc(sem_a4, 16)
            for t in (h1f, h2f):
                nc.gpsimd.memset(t[:, :], 0.0).then_inc(gp_sem, 1)
            nc.gpsimd.dma_start(t5[:, :], d_5[:, :]).then_inc(sem_a5, 16)

        @block.sync
        def _(sync):
            sync.wait_ge(dve_sem, 2 * nsweep)
            sync.dma_start(xt[:, :], state_in(nsweep)[0:FD, 0:S].bitcast(_DT)).then_inc(sem_w, 16)
            sync.wait_ge(sem_w, 48)

        @block.tensor
        def _(tensor):
            # dep-free filler matmul: keeps the PE pipeline continuously busy
            # across sem waits so it ramps to (and stays at) full p-state
            def dmy(n):
                for _ in range(n):
                    nc.tensor.matmul(pdmy[:, :], t1[0:1, 0:1], t1[0:1, 0:16], start=True, stop=True)

            tensor.wait_ge(sem_w, 16)
            first_t5 = next((i for i, c in enumerate(plan) if c != "r"), None)
            for j, prec in enumerate(plan):
                om = rops if mlp_r(prec) else fops
                oi = rops if int_r(prec) else fops
                mst = state_in(j)
                if j > 0:
                    tensor.wait_ge(dve_sem, 2 * j)
                if j == first_t5:
                    tensor.wait_ge(sem_b, 16)
                    tensor.wait_ge(sem_a5, 16)
                nc.tensor.matmul(p1[:, :], om["w1"], mst[:, :], start=True, stop=True).then_inc(pe_sem, 1)
                dmy(NDMY)
                if j == 0:
                    tensor.wait_ge(sem_w, 32)
                tensor.wait_ge(act_sem, 2 * j + 1)
                nc.tensor.matmul(p2[:, :], om["w2"], om["h1"][:, :], start=True, stop=True).then_inc(pe_sem, 1)
                dmy(NDMY)
                tensor.wait_ge(act_sem, 2 * j + 2)
                nc.tensor.matmul(pf[:, 0:FD], om["h2"][:, 0:K1], om["w3"], start=True, stop=True).then_inc(pe_sem, 1)
                nc.tensor.matmul(pf[0:K2, FD:2 * FD], om["h2"][:, K1:S], om["w3"], start=True, stop=True).then_inc(pe_sem, 1)
                dmy(NDMY)
                tensor.wait_ge(dve_sem, 2 * j + 1)
                if j == 0:
                    tensor.wait_ge(sem_a3, 16)
                    tensor.wait_ge(sem_a4, 16)
                nc.tensor.matmul(px[:, :], oi["fab"][:, 0:FD], oi["ata"], start=True, stop=False).then_inc(pe_sem, 1)
                nc.tensor.matmul(px[:, :], oi["fab"][0:K2, FD:2 * FD], oi["atb"], start=False, stop=True).then_inc(pe_sem, 1)
                dmy(NDMY)

        @block.scalar
        def _(scalar):
            nc.scalar.dma_start(t2[:, :], d_2[:, :]).then_inc(sem_b, 16)
            nc.scalar.dma_start(ata_r[:, :], d_3[:, :]).then_inc(sem_a3, 16)
            # dep-free warm-up: zero a scratch tile, tanh it -> loads the
            # Tanh table while the input DMAs are still in flight.
            nc.scalar.mul(warm[:, :], warm[:, :], 0.0)
            nc.scalar.activation(warm[:, :], warm[:, :], tanh)
            scalar.wait_ge(gp_sem, 2)
            scalar.wait_ge(sem_b, 16)
            first_fmlp = next((i for i, c in enumerate(plan) if not mlp_r(c)), None)
            for j, prec in enumerate(plan):
                o = rops if mlp_r(prec) else fops
                if j == first_fmlp:
                    scalar.wait_ge(gp_sem, 4)
                scalar.wait_ge(pe_sem, 6 * j + 1)
                nc.scalar.activation(o["h1"][:, 0:S], p1[:, 0:S], tanh, bias=b1t).then_inc(act_sem, 1)
                scalar.wait_ge(pe_sem, 6 * j + 2)
                nc.scalar.activation(o["h2"][:, 0:S], p2[:, 0:S], tanh, bias=b2t).then_inc(act_sem, 1)

        @block.vector
        def _(vector):
            vector.wait_ge(sem_b, 16)
            sttf_guarded = False
            for j, prec in enumerate(plan):
                oi = rops if int_r(prec) else fops
                vector.wait_ge(pe_sem, 6 * j + 4)
                nc.vector.tensor_copy(oi["fab"][:, :], pf[:, :]).then_inc(dve_sem, 1)
                vector.wait_ge(pe_sem, 6 * j + 6)
                if state_in(j + 1) is sttf and not sttf_guarded:
                    # sttf lives in the D5 tile; don't write the state row
                    # band until that transfer has landed
                    vector.wait_ge(sem_a5, 16)
                    sttf_guarded = True
                nc.vector.tensor_add(state_in(j + 1)[0:FD, 0:S], px[:, 0:S], dm[:, 0:S]).then_inc(dve_sem, 1)

    return nc


_NC_CACHE = {}


def _get_nc(plan=PLAN):
    if plan not in _NC_CACHE:
        _NC_CACHE[plan] = _build_nc(plan)
    return _NC_CACHE[plan]


def _host_prep(x0, treatments, ts, W1, b1, W2, b2, W3, b3):
    ts64 = ts.astype(np.float64)
    tr64 = treatments.astype(np.float64)
    x064 = x0.reshape(FD).astype(np.float64)
    b364 = b3.astype(np.float64)

    # resampled treatments at grid + midpoints (linear within an interval)
    TR = np.zeros((S, TD), np.float64)
    TR[0::2] = tr64
    TR[1::2] = 0.5 * (tr64[:-1] + tr64[1:])

    # cumulative composite-Simpson quadrature matrix A [S,S]:
    # (A @ F)[s] ~= \int_{t_0}^{t_s} f dt  for F sampled at the S times.
    h = np.diff(ts64)
    A = np.zeros((S, S), np.float64)
    row = np.zeros(S, np.float64)
    for k in range(T - 1):
        mrow = row.copy()
        mrow[2 * k : 2 * k + 3] += h[k] * np.array([5.0, 8.0, -1.0]) / 24.0
        A[2 * k + 1] = mrow
        row[2 * k : 2 * k + 3] += h[k] * np.array([1.0, 4.0, 1.0]) / 6.0
        A[2 * k + 2] = row

    # D[j, s] = x0[j] + b3[j] * rowsum(A)[s]  (folds both the x0 offset and
    # the b3 bias contribution of the last MLP layer into one constant).
    D = x064[:, None] + b364[:, None] * A.sum(axis=1)[None, :]

    AT = A.T
    D1 = np.zeros((HD, D1W), np.float64)
    D1[0:FD, _D1_ST:_D1_ST + S] = x064[:, None]
    D1[FD:IN_DIM, _D1_ST:_D1_ST + S] = TR.T
    D1[0:IN_DIM, _D1_W1:_D1_W1 + HD] = W1
    D1[0:HD, _D1_W2:_D1_W2 + HD] = W2
    D1[0:HD, _D1_W3:_D1_W3 + FD] = W3

    D2 = np.zeros((HD, D2W), np.float64)
    D2[0:HD, _D2_B1] = b1
    D2[0:HD, _D2_B2] = b2
    D2[0:FD, _D2_DM:_D2_DM + S] = D

    D3 = np.zeros((K1, SP), np.float64)
    D3[:, 0:S] = AT[0:K1]
    D4 = np.zeros((K2, SP), np.float64)
    D4[:, 0:S] = AT[K1:S]
    D5 = np.zeros((K1, D5W), np.float64)
    D5[0:K1, _D5_ATA:_D5_ATA + S] = AT[0:K1]
    D5[0:K2, _D5_ATB:_D5_ATB + S] = AT[K1:S]
    D5[0:IN_DIM, _D5_W1:_D5_W1 + HD] = W1
    D5[0:HD, _D5_W2:_D5_W2 + HD] = W2
    D5[0:HD, _D5_W3:_D5_W3 + FD] = W3
    D5[0:FD, _D5_ST:_D5_ST + S] = x064[:, None]
    D5[FD:IN_DIM, _D5_ST:_D5_ST + S] = TR.T

    f32 = lambda a: np.ascontiguousarray(a, dtype=np.float32)
    return {"d1": f32(D1), "d2": f32(D2), "d3": f32(D3), "d4": f32(D4), "d5": f32(D5)}


def kernel(x0, treatments, ts, W1, b1, W2, b2, W3, b3, _results=None, _plan=PLAN):
    in_map = _host_prep(x0, treatments, ts, W1, b1, W2, b2, W3, b3)
    nc = _get_nc(_plan)
    res = bass_utils.run_bass_kernel_spmd(
        nc, [in_map] * N_CORES, core_ids=list(range(N_CORES))
    )
    if _results is not None:
        _results.append(res)
    xt = res.results[0]["xt"]  # [FD, S]
    out = xt.T[0::2].reshape(T, 1, FD)
    return np.ascontiguousarray(out, dtype=np.float32)



# revision 3
# speedup vs baseline: 1.8466x; 1.8466x over previous
r"""Trainium2 Bass kernel for the CounterfactualODEModel problem.

Reference computes an adaptive dopri5 solve of
    dx/dt = MLP(concat(x, tr(t))),  tr = piecewise-linear treatments,
evaluated at the T=100 grid times.  Instead of replaying the sequential
solver on device, this kernel uses a parallel-in-time Picard iteration on
the integral form  x(t) = x0 + \int_0^t f(x(s), s) ds:

  - sample times = exactly the T=100 grid points; within one interval the
    treatments are linear so the integrand is smooth there, and composite
    trapezoid gives a quadrature fixed point ~1.4e-4 from the dopri5
    reference -- far inside the 2e-2 gate,
  - each sweep evaluates the MLP at all T times as batched matmuls and
    applies the cumulative-trapezoid matrix A (built on host from ts):
        X <- x0 + A @ f(X),
    with the x0 / b3 constant terms folded into two extra contraction rows
    of the integration matmul, so the updated state drops out of a single
    PSUM accumulation,
  - the iteration contracts ~10-25x per sweep; NSWEEP=2 sweeps land at
    ~2.2e-3 relative error (measured against the f32 dopri5 reference).

Everything runs in bf16 (1 PE cycle/row at ANY moving-dim size, unlike
f32r which needs >=256), so no free-dim padding is needed: all matmuls
move exactly 100 (or 32) rows.  PSUM accumulation stays fp32 and the
final state is DMA'd to HBM straight out of PSUM.

Implementation notes:
  - raw Bass (explicit engine streams + semaphores): the walrus build in
    this environment rejects instructions carrying more than one attached
    sync-wait, so standalone wait_ge instructions are used throughout,
  - inputs ride two parallel DMAs (state+weights+biases | quadrature),
    issued by gpsimd and sync so the MLP matmuls of sweep 0 start as soon
    as the first ~34KB lands; a dep-free tanh on scratch data preloads
    the ACT table under the input DMAs,
  - no memsets: every SBUF region a matmul reads is fully written either
    by a zero-padded host buffer or by a producer instruction.

The whole state is tiny (100 x 36), so the problem is replicated on all
8 cores (no useful tensor/batch parallelism exists for one trajectory);
core 0's output is returned.
"""

import numpy as np
import ml_dtypes

from contextlib import ExitStack

import concourse.bass as bass
import concourse.mybir as mybir
from concourse import bass_utils

T = 100
FD = 32   # feature dim
TD = 4    # treatment dim
HD = 64   # hidden dim
IN_DIM = FD + TD
NSWEEP = 2
PLAN = NSWEEP  # kept for test.py / prof.py compatibility
N_CORES = 8

_F32 = mybir.dt.float32
_BF = mybir.dt.bfloat16

# d ([102, 396] bf16) column layout:
#   ST | W1 | B1 | B2 | W2 | W3 || ATA | FAB-const
_C_ST = 0
_C_W1 = _C_ST + T
_C_B1 = _C_W1 + HD          # fp32 [64,1] as 2 bf16 cols (byte off 4-aligned)
_C_B2 = _C_B1 + 2
_C_W2 = _C_B2 + 2
_C_W3 = _C_W2 + HD
_C1 = _C_W3 + FD            # end of DMA1 (rows 0:64)
_C_ATA = _C1                # [102, 100]: rows 0:100 = A^T, 100 = ones, 101 = rowsum(A)
_C_FAB = _C_ATA + T         # [102, 32]: rows 100:102 = [x0; b3], rows 0:100 live
DW = _C_FAB + FD
DP = T + 2                  # partition count (100 samples + 2 const rows)


class _LeanBlock(bass.BassBlock):
    """Block whose exit skips the all-engine EVSEM butterfly: engines just
    drain and end.  Output integrity is guaranteed by the sync stream's
    final wait on the output-DMA semaphore; semaphores are re-cleared by
    the preamble on every execution."""

    def __exit__(self, exc_type, exc_val, exc_tb):
        if exc_type is not None:
            return
        for engine, last_body in self.last_body.items():
            with self.bass.body(
                last_body, parent=self.bass.cur_bb, allow_existing_parent=True
            ):
                engine.br(self.end_bb)
        self.bass.switch_bb(self.end_bb)
        gpsimd_type = self.bass.gpsimd.engine
        for eng_type, eng in self.bass.engines.items():
            if eng_type == gpsimd_type:
                continue
            d = mybir.InstDrain(
                name=self.bass.get_next_instruction_name(),
                ins=[],
                outs=[],
                bass_is_fusable=False,
            )
            d.engine = eng_type
            eng.add_instruction(d)


def _build_nc(nsweep=NSWEEP):
    nc = bass.Bass(trn_type="TRN2", monotonic_sem_count=0, enable_partition_id=False)
    d = nc.dram_tensor("d", [DP, DW], _BF, kind="ExternalInput")
    xt = nc.dram_tensor("xt", [FD, T], _F32, kind="ExternalOutput")

    tanh = mybir.ActivationFunctionType.Tanh

    with ExitStack() as ctx:
        sb = lambda nm, shape, dt: ctx.enter_context(nc.sbuf_tensor(nm, shape, dt))
        ps = lambda nm, shape: ctx.enter_context(nc.psum_tensor(nm, shape, _F32))
        sem = lambda nm: ctx.enter_context(nc.semaphore(nm))
        t = sb("t_d", [DP, DW], _BF)
        h1 = sb("t_h1", [HD, T], _BF)
        h2 = sb("t_h2", [HD, T], _BF)
        warm = sb("t_warm", [HD, 1], _F32)
        xt_sb = sb("t_xt", [FD, T], _F32)
        p1 = ps("t_p1", [HD, T])
        p2 = ps("t_p2", [HD, T])
        pf = ps("t_pf", [T, FD])
        px = ps("t_px", [FD, T])
        sem_d1 = sem("sem_d1")
        sem_d2 = sem("sem_d2")
        pe_sem = sem("sem_pe")
        act_sem = sem("sem_act")
        dve_sem = sem("sem_dve")
        out_sem = sem("sem_out")

        st = t[0:IN_DIM, _C_ST:_C_ST + T]
        w1 = t[0:IN_DIM, _C_W1:_C_W1 + HD]
        w2 = t[0:HD, _C_W2:_C_W2 + HD]
        w3 = t[0:HD, _C_W3:_C_W3 + FD]
        b1 = t[0:HD, _C_B1:_C_B1 + 2].bitcast(_F32)
        b2 = t[0:HD, _C_B2:_C_B2 + 2].bitcast(_F32)
        ata = t[0:DP, _C_ATA:_C_ATA + T]
        fab = t[0:DP, _C_FAB:_C_FAB + FD]
        fabv = t[0:T, _C_FAB:_C_FAB + FD]   # per-sweep DVE write region

        block = ctx.enter_context(_LeanBlock(nc, 'blk'))

        # semaphore plan (DMA sems inc by 16 at transfer completion):
        #   sem_d1: DMA1 (state+weights+biases) = 16
        #   sem_d2: DMA2 (quadrature + fab consts) = 16
        #   pe_sem:  4 matmuls/sweep -> 4j+k after k-th matmul of sweep j
        #   act_sem: 2 tanhs/sweep   -> 2j+k
        #   dve_sem: 2 copies/sweep (fab, state), no state copy last sweep
        #   out_sem: output DMA (PSUM px -> HBM) = 16

        @block.gpsimd
        def _(gpsimd):
            nc.gpsimd.dma_start(t[0:HD, 0:_C1], d[0:HD, 0:_C1]).then_inc(sem_d1, 16)

        @block.sync
        def _(sync):
            sync.dma_start(t[0:DP, _C1:DW], d[0:DP, _C1:DW]).then_inc(sem_d2, 16)
            sync.wait_ge(dve_sem, 2 * nsweep)
            sync.dma_start(xt[:, :], xt_sb[:, :]).then_inc(out_sem, 16)
            sync.wait_ge(out_sem, 16)

        @block.tensor
        def _(tensor):
            tensor.wait_ge(sem_d1, 16)
            for j in range(nsweep):
                if j > 0:
                    tensor.wait_ge(dve_sem, 2 * j)
                nc.tensor.matmul(p1[:, :], w1, st, start=True, stop=True).then_inc(pe_sem, 1)
                tensor.wait_ge(act_sem, 2 * j + 1)
                nc.tensor.matmul(p2[:, :], w2, h1[:, :], start=True, stop=True).then_inc(pe_sem, 1)
                tensor.wait_ge(act_sem, 2 * j + 2)
                nc.tensor.matmul(pf[:, :], h2[:, :], w3, start=True, stop=True).then_inc(pe_sem, 1)
                tensor.wait_ge(dve_sem, 2 * j + 1)
                if j == 0:
                    tensor.wait_ge(sem_d2, 16)
                nc.tensor.matmul(px[:, :], fab, ata, start=True, stop=True).then_inc(pe_sem, 1)

        @block.scalar
        def _(scalar):
            # dep-free warm-up: zero a scratch tile, tanh it -> loads the
            # Tanh table while the input DMAs are still in flight.
            nc.scalar.mul(warm[:, :], warm[:, :], 0.0)
            nc.scalar.activation(warm[:, :], warm[:, :], tanh)
            for j in range(nsweep):
                scalar.wait_ge(pe_sem, 4 * j + 1)
                nc.scalar.activation(h1[:, :], p1[:, :], tanh, bias=b1).then_inc(act_sem, 1)
                scalar.wait_ge(pe_sem, 4 * j + 2)
                nc.scalar.activation(h2[:, :], p2[:, :], tanh, bias=b2).then_inc(act_sem, 1)

        @block.vector
        def _(vector):
            for j in range(nsweep):
                vector.wait_ge(pe_sem, 4 * j + 3)
                nc.vector.tensor_copy(fabv, pf[:, :]).then_inc(dve_sem, 1)
                vector.wait_ge(pe_sem, 4 * j + 4)
                if j < nsweep - 1:
                    nc.vector.tensor_copy(st[0:FD, :], px[:, :]).then_inc(dve_sem, 1)
                else:
                    nc.vector.tensor_copy(xt_sb[:, :], px[:, :]).then_inc(dve_sem, 1)

    return nc


_NC_CACHE = {}


def _get_nc(nsweep=NSWEEP):
    nsweep = int(nsweep)
    if nsweep not in _NC_CACHE:
        _NC_CACHE[nsweep] = _build_nc(nsweep)
    return _NC_CACHE[nsweep]


def _host_prep(x0, treatments, ts, W1, b1, W2, b2, W3, b3):
    ts64 = ts.astype(np.float64)
    x032 = np.ascontiguousarray(x0.reshape(FD), dtype=np.float32)

    # cumulative trapezoid quadrature matrix A [T,T]:
    # (A @ F)[t] ~= \int_{t_0}^{t_t} f dt  for F sampled at the grid times.
    h = np.diff(ts64)
    A = np.zeros((T, T), np.float64)
    for k in range(T - 1):
        A[k + 1] = A[k]
        A[k + 1, k] += h[k] / 2
        A[k + 1, k + 1] += h[k] / 2

    D = np.zeros((DP, DW), dtype=ml_dtypes.bfloat16)
    D[0:FD, _C_ST:_C_ST + T] = x032[:, None]
    D[FD:IN_DIM, _C_ST:_C_ST + T] = treatments.T
    D[0:IN_DIM, _C_W1:_C_W1 + HD] = W1
    D[0:HD, _C_W2:_C_W2 + HD] = W2
    D[0:HD, _C_W3:_C_W3 + FD] = W3
    D[0:T, _C_ATA:_C_ATA + T] = A.T
    D[T, _C_ATA:_C_ATA + T] = 1.0
    D[T + 1, _C_ATA:_C_ATA + T] = A.sum(axis=1)
    D[T, _C_FAB:_C_FAB + FD] = x032
    D[T + 1, _C_FAB:_C_FAB + FD] = b3
    # biases stay exact fp32, stored as raw bf16-bit pairs
    u16 = D.view(np.uint16)
    u16[0:HD, _C_B1:_C_B1 + 2] = np.ascontiguousarray(
        b1.reshape(HD, 1), dtype=np.float32).view(np.uint16)
    u16[0:HD, _C_B2:_C_B2 + 2] = np.ascontiguousarray(
        b2.reshape(HD, 1), dtype=np.float32).view(np.uint16)
    return {"d": D}


def kernel(x0, treatments, ts, W1, b1, W2, b2, W3, b3, _results=None, _plan=NSWEEP):
    in_map = _host_prep(x0, treatments, ts, W1, b1, W2, b2, W3, b3)
    nc = _get_nc(_plan)
    res = bass_utils.run_bass_kernel_spmd(
        nc, [in_map] * N_CORES, core_ids=list(range(N_CORES))
    )
    if _results is not None:
        _results.append(res)
    xt = res.results[0]["xt"]  # [FD, T]
    out = xt.T.reshape(T, 1, FD)
    return np.ascontiguousarray(out, dtype=np.float32)


# revision 4
# speedup vs baseline: 1.8651x; 1.0100x over previous
r"""Trainium2 Bass kernel for the CounterfactualODEModel problem.

Reference computes an adaptive dopri5 solve of
    dx/dt = MLP(concat(x, tr(t))),  tr = piecewise-linear treatments,
evaluated at the T=100 grid times.  Instead of replaying the sequential
solver on device, this kernel uses a parallel-in-time Picard iteration on
the integral form  x(t) = x0 + \int_0^t f(x(s), s) ds:

  - sample times = exactly the T=100 grid points; within one interval the
    treatments are linear so the integrand is smooth there, and composite
    trapezoid gives a quadrature fixed point ~1.4e-4 from the dopri5
    reference -- far inside the 2e-2 gate,
  - each sweep evaluates the MLP at all T times as batched matmuls and
    applies the cumulative-trapezoid matrix A (built on host from ts):
        X <- x0 + A @ f(X),
    with the x0 / b3 constant terms folded into two extra contraction rows
    of the integration matmul, so the updated state drops out of a single
    PSUM accumulation,
  - the iteration contracts ~10-25x per sweep; NSWEEP=2 sweeps land at
    ~2.2e-3 relative error (measured against the f32 dopri5 reference).

Everything runs in bf16 (1 PE cycle/row at ANY moving-dim size, unlike
f32r which needs >=256), so no free-dim padding is needed: all matmuls
move exactly 100 (or 32) rows.  PSUM accumulation stays fp32 and the
final state is DMA'd to HBM straight out of PSUM.

Implementation notes:
  - raw Bass (explicit engine streams + semaphores): the walrus build in
    this environment rejects instructions carrying more than one attached
    sync-wait, so standalone wait_ge instructions are used throughout,
  - inputs ride two parallel DMAs (state+weights+biases | quadrature),
    issued by sync and scalar (the two engines that enter the block body
    first) so the MLP matmuls of sweep 0 start as soon as the first ~34KB
    lands; a dep-free tanh on scratch data preloads the ACT table under
    the input DMAs,
  - no memsets: every SBUF region a matmul reads is fully written either
    by a zero-padded host buffer or by a producer instruction.

The whole state is tiny (100 x 36), so the problem is replicated on all
8 cores (no useful tensor/batch parallelism exists for one trajectory);
core 0's output is returned.
"""

import numpy as np
import ml_dtypes

from contextlib import ExitStack

import concourse.bass as bass
import concourse.mybir as mybir
from concourse import bass_utils

T = 100
FD = 32   # feature dim
TD = 4    # treatment dim
HD = 64   # hidden dim
IN_DIM = FD + TD
NSWEEP = 2
PLAN = NSWEEP  # kept for test.py / prof.py compatibility
N_CORES = 8

_F32 = mybir.dt.float32
_BF = mybir.dt.bfloat16

# d ([102, 396] bf16) column layout:
#   ST | W1 | B1 | B2 | W2 | W3 || ATA | FAB-const
_C_ST = 0
_C_W1 = _C_ST + T
_C_B1 = _C_W1 + HD          # fp32 [64,1] as 2 bf16 cols (byte off 4-aligned)
_C_B2 = _C_B1 + 2
_C_W2 = _C_B2 + 2
_C_W3 = _C_W2 + HD
_C1 = _C_W3 + FD            # end of DMA1 (rows 0:64)
_C_ATA = _C1                # [102, 100]: rows 0:100 = A^T, 100 = ones, 101 = rowsum(A)
_C_FAB = _C_ATA + T         # [102, 32]: rows 100:102 = [x0; b3], rows 0:100 live
DW = _C_FAB + FD
DP = T + 2                  # partition count (100 samples + 2 const rows)


class _LeanBlock(bass.BassBlock):
    """Block whose exit skips the all-engine EVSEM butterfly: engines just
    drain and end.  Output integrity is guaranteed by the sync stream's
    final wait on the output-DMA semaphore; semaphores are re-cleared by
    the preamble on every execution."""

    def __exit__(self, exc_type, exc_val, exc_tb):
        if exc_type is not None:
            return
        for engine, last_body in self.last_body.items():
            with self.bass.body(
                last_body, parent=self.bass.cur_bb, allow_existing_parent=True
            ):
                engine.br(self.end_bb)
        self.bass.switch_bb(self.end_bb)
        gpsimd_type = self.bass.gpsimd.engine
        for eng_type, eng in self.bass.engines.items():
            if eng_type == gpsimd_type:
                continue
            d = mybir.InstDrain(
                name=self.bass.get_next_instruction_name(),
                ins=[],
                outs=[],
                bass_is_fusable=False,
            )
            d.engine = eng_type
            eng.add_instruction(d)


def _build_nc(nsweep=NSWEEP):
    nc = bass.Bass(trn_type="TRN2", monotonic_sem_count=0, enable_partition_id=False)
    d = nc.dram_tensor("d", [DP, DW], _BF, kind="ExternalInput")
    xt = nc.dram_tensor("xt", [FD, T], _F32, kind="ExternalOutput")

    tanh = mybir.ActivationFunctionType.Tanh

    with ExitStack() as ctx:
        sb = lambda nm, shape, dt: ctx.enter_context(nc.sbuf_tensor(nm, shape, dt))
        ps = lambda nm, shape: ctx.enter_context(nc.psum_tensor(nm, shape, _F32))
        sem = lambda nm: ctx.enter_context(nc.semaphore(nm))
        t = sb("t_d", [DP, DW], _BF)
        h1 = sb("t_h1", [HD, T], _BF)
        h2 = sb("t_h2", [HD, T], _BF)
        warm = sb("t_warm", [HD, 1], _F32)
        xt_sb = sb("t_xt", [FD, T], _F32)
        p1 = ps("t_p1", [HD, T])
        p2 = ps("t_p2", [HD, T])
        pf = ps("t_pf", [T, FD])
        px = ps("t_px", [FD, T])
        sem_d1 = sem("sem_d1")
        sem_d2 = sem("sem_d2")
        pe_sem = sem("sem_pe")
        act_sem = sem("sem_act")
        dve_sem = sem("sem_dve")
        out_sem = sem("sem_out")

        st = t[0:IN_DIM, _C_ST:_C_ST + T]
        w1 = t[0:IN_DIM, _C_W1:_C_W1 + HD]
        w2 = t[0:HD, _C_W2:_C_W2 + HD]
        w3 = t[0:HD, _C_W3:_C_W3 + FD]
        b1 = t[0:HD, _C_B1:_C_B1 + 2].bitcast(_F32)
        b2 = t[0:HD, _C_B2:_C_B2 + 2].bitcast(_F32)
        ata = t[0:DP, _C_ATA:_C_ATA + T]
        fab = t[0:DP, _C_FAB:_C_FAB + FD]
        fabv = t[0:T, _C_FAB:_C_FAB + FD]   # per-sweep DVE write region

        block = ctx.enter_context(_LeanBlock(nc, 'blk'))

        # semaphore plan (DMA sems inc by 16 at transfer completion):
        #   sem_d1: DMA1 (state+weights+biases) = 16
        #   sem_d2: DMA2 (quadrature + fab consts) = 16
        #   pe_sem:  4 matmuls/sweep -> 4j+k after k-th matmul of sweep j
        #   act_sem: 2 tanhs/sweep   -> 2j+k
        #   dve_sem: 2 copies/sweep (fab, state), no state copy last sweep
        #   out_sem: output DMA (PSUM px -> HBM) = 16

        @block.sync
        def _(sync):
            sync.dma_start(t[0:HD, 0:_C1], d[0:HD, 0:_C1]).then_inc(sem_d1, 16)
            sync.wait_ge(dve_sem, 2 * nsweep)
            sync.dma_start(xt[:, :], xt_sb[:, :]).then_inc(out_sem, 16)
            sync.wait_ge(out_sem, 16)

        @block.tensor
        def _(tensor):
            tensor.wait_ge(sem_d1, 16)
            for j in range(nsweep):
                if j > 0:
                    tensor.wait_ge(dve_sem, 2 * j)
                nc.tensor.matmul(p1[:, :], w1, st, start=True, stop=True).then_inc(pe_sem, 1)
                tensor.wait_ge(act_sem, 2 * j + 1)
                nc.tensor.matmul(p2[:, :], w2, h1[:, :], start=True, stop=True).then_inc(pe_sem, 1)
                tensor.wait_ge(act_sem, 2 * j + 2)
                nc.tensor.matmul(pf[:, :], h2[:, :], w3, start=True, stop=True).then_inc(pe_sem, 1)
                tensor.wait_ge(dve_sem, 2 * j + 1)
                if j == 0:
                    tensor.wait_ge(sem_d2, 16)
                nc.tensor.matmul(px[:, :], fab, ata, start=True, stop=True).then_inc(pe_sem, 1)

        @block.scalar
        def _(scalar):
            nc.scalar.dma_start(t[0:DP, _C1:DW], d[0:DP, _C1:DW]).then_inc(sem_d2, 16)
            # dep-free warm-up: zero a scratch tile, tanh it -> loads the
            # Tanh table while the input DMAs are still in flight.
            nc.scalar.mul(warm[:, :], warm[:, :], 0.0)
            nc.scalar.activation(warm[:, :], warm[:, :], tanh)
            for j in range(nsweep):
                scalar.wait_ge(pe_sem, 4 * j + 1)
                nc.scalar.activation(h1[:, :], p1[:, :], tanh, bias=b1).then_inc(act_sem, 1)
                scalar.wait_ge(pe_sem, 4 * j + 2)
                nc.scalar.activation(h2[:, :], p2[:, :], tanh, bias=b2).then_inc(act_sem, 1)

        @block.vector
        def _(vector):
            for j in range(nsweep):
                vector.wait_ge(pe_sem, 4 * j + 3)
                nc.vector.tensor_copy(fabv, pf[:, :]).then_inc(dve_sem, 1)
                vector.wait_ge(pe_sem, 4 * j + 4)
                if j < nsweep - 1:
                    nc.vector.tensor_copy(st[0:FD, :], px[:, :]).then_inc(dve_sem, 1)
                else:
                    nc.vector.tensor_copy(xt_sb[:, :], px[:, :]).then_inc(dve_sem, 1)

    return nc


_NC_CACHE = {}


def _get_nc(nsweep=NSWEEP):
    nsweep = int(nsweep)
    if nsweep not in _NC_CACHE:
        _NC_CACHE[nsweep] = _build_nc(nsweep)
    return _NC_CACHE[nsweep]


def _host_prep(x0, treatments, ts, W1, b1, W2, b2, W3, b3):
    ts64 = ts.astype(np.float64)
    x032 = np.ascontiguousarray(x0.reshape(FD), dtype=np.float32)

    # cumulative trapezoid quadrature matrix A [T,T]:
    # (A @ F)[t] ~= \int_{t_0}^{t_t} f dt  for F sampled at the grid times.
    h = np.diff(ts64)
    A = np.zeros((T, T), np.float64)
    for k in range(T - 1):
        A[k + 1] = A[k]
        A[k + 1, k] += h[k] / 2
        A[k + 1, k + 1] += h[k] / 2

    D = np.zeros((DP, DW), dtype=ml_dtypes.bfloat16)
    D[0:FD, _C_ST:_C_ST + T] = x032[:, None]
    D[FD:IN_DIM, _C_ST:_C_ST + T] = treatments.T
    D[0:IN_DIM, _C_W1:_C_W1 + HD] = W1
    D[0:HD, _C_W2:_C_W2 + HD] = W2
    D[0:HD, _C_W3:_C_W3 + FD] = W3
    D[0:T, _C_ATA:_C_ATA + T] = A.T
    D[T, _C_ATA:_C_ATA + T] = 1.0
    D[T + 1, _C_ATA:_C_ATA + T] = A.sum(axis=1)
    D[T, _C_FAB:_C_FAB + FD] = x032
    D[T + 1, _C_FAB:_C_FAB + FD] = b3
    # biases stay exact fp32, stored as raw bf16-bit pairs
    u16 = D.view(np.uint16)
    u16[0:HD, _C_B1:_C_B1 + 2] = np.ascontiguousarray(
        b1.reshape(HD, 1), dtype=np.float32).view(np.uint16)
    u16[0:HD, _C_B2:_C_B2 + 2] = np.ascontiguousarray(
        b2.reshape(HD, 1), dtype=np.float32).view(np.uint16)
    return {"d": D}


def kernel(x0, treatments, ts, W1, b1, W2, b2, W3, b3, _results=None, _plan=NSWEEP):
    in_map = _host_prep(x0, treatments, ts, W1, b1, W2, b2, W3, b3)
    nc = _get_nc(_plan)
    res = bass_utils.run_bass_kernel_spmd(
        nc, [in_map] * N_CORES, core_ids=list(range(N_CORES))
    )
    if _results is not None:
        _results.append(res)
    xt = res.results[0]["xt"]  # [FD, T]
    out = xt.T.reshape(T, 1, FD)
    return np.ascontiguousarray(out, dtype=np.float32)


# revision 5
# speedup vs baseline: 1.9051x; 1.0215x over previous
r"""Trainium2 Bass kernel for the CounterfactualODEModel problem.

Reference computes an adaptive dopri5 solve of
    dx/dt = MLP(concat(x, tr(t))),  tr = piecewise-linear treatments,
evaluated at the T=100 grid times.  Instead of replaying the sequential
solver on device, this kernel uses a parallel-in-time Picard iteration on
the integral form  x(t) = x0 + \int_0^t f(x(s), s) ds:

  - sample times = exactly the T=100 grid points; within one interval the
    treatments are linear so the integrand is smooth there, and composite
    trapezoid gives a quadrature fixed point ~1.4e-4 from the dopri5
    reference -- far inside the 2e-2 gate,
  - each sweep evaluates the MLP at all T times as batched matmuls and
    applies the cumulative-trapezoid matrix A (built on host from ts):
        X <- x0 + A @ f(X),
    with the x0 / b3 constant terms folded into two extra contraction rows
    of the integration matmul, so the updated state drops out of a single
    PSUM accumulation,
  - the iteration contracts ~10-25x per sweep; NSWEEP=2 sweeps land at
    ~2.2e-3 relative error (measured against the f32 dopri5 reference).

Everything runs in bf16 (1 PE cycle/row at ANY moving-dim size, unlike
f32r which needs >=256), so no free-dim padding is needed: all matmuls
move exactly 100 (or 32) rows.  PSUM accumulation stays fp32 and the
final state is DMA'd to HBM straight out of PSUM.

Implementation notes:
  - raw Bass (explicit engine streams + semaphores): the walrus build in
    this environment rejects instructions carrying more than one attached
    sync-wait, so standalone wait_ge instructions are used throughout,
  - inputs ride two parallel DMAs: the critical state+weights+biases
    chunk is issued by sync (the first engine to enter the block body),
    the quadrature chunk by gpsimd; a dep-free tanh on scratch data
    preloads the ACT table under the input DMAs,
  - no memsets: every SBUF region a matmul reads is fully written either
    by a zero-padded host buffer or by a producer instruction.

The whole state is tiny (100 x 36), so the problem is replicated on all
8 cores (no useful tensor/batch parallelism exists for one trajectory);
core 0's output is returned.
"""

import numpy as np
import ml_dtypes

from contextlib import ExitStack

import concourse.bass as bass
import concourse.mybir as mybir
from concourse import bass_utils

T = 100
FD = 32   # feature dim
TD = 4    # treatment dim
HD = 64   # hidden dim
IN_DIM = FD + TD
NSWEEP = 2
PLAN = NSWEEP  # kept for test.py / prof.py compatibility
N_CORES = 8

_F32 = mybir.dt.float32
_BF = mybir.dt.bfloat16

# d ([102, 396] bf16) column layout:
#   ST | W1 | B1 | B2 | W2 | W3 || ATA | FAB-const
_C_ST = 0
_C_W1 = _C_ST + T
_C_B1 = _C_W1 + HD          # fp32 [64,1] as 2 bf16 cols (byte off 4-aligned)
_C_B2 = _C_B1 + 2
_C_W2 = _C_B2 + 2
_C_W3 = _C_W2 + HD
_C1 = _C_W3 + FD            # end of DMA1 (rows 0:64)
_C_ATA = _C1                # [102, 100]: rows 0:100 = A^T, 100 = ones, 101 = rowsum(A)
_C_FAB = _C_ATA + T         # [102, 32]: rows 100:102 = [x0; b3], rows 0:100 live
DW = _C_FAB + FD
DP = T + 2                  # partition count (100 samples + 2 const rows)


class _LeanBlock(bass.BassBlock):
    """Block whose exit skips the all-engine EVSEM butterfly: engines just
    drain and end.  Output integrity is guaranteed by the sync stream's
    final wait on the output-DMA semaphore; semaphores are re-cleared by
    the preamble on every execution."""

    def __exit__(self, exc_type, exc_val, exc_tb):
        if exc_type is not None:
            return
        for engine, last_body in self.last_body.items():
            with self.bass.body(
                last_body, parent=self.bass.cur_bb, allow_existing_parent=True
            ):
                engine.br(self.end_bb)
        self.bass.switch_bb(self.end_bb)
        gpsimd_type = self.bass.gpsimd.engine
        for eng_type, eng in self.bass.engines.items():
            if eng_type == gpsimd_type:
                continue
            d = mybir.InstDrain(
                name=self.bass.get_next_instruction_name(),
                ins=[],
                outs=[],
                bass_is_fusable=False,
            )
            d.engine = eng_type
            eng.add_instruction(d)


def _build_nc(nsweep=NSWEEP):
    nc = bass.Bass(trn_type="TRN2", monotonic_sem_count=0, enable_partition_id=False)
    d = nc.dram_tensor("d", [DP, DW], _BF, kind="ExternalInput")
    xt = nc.dram_tensor("xt", [FD, T], _F32, kind="ExternalOutput")

    tanh = mybir.ActivationFunctionType.Tanh

    with ExitStack() as ctx:
        sb = lambda nm, shape, dt: ctx.enter_context(nc.sbuf_tensor(nm, shape, dt))
        ps = lambda nm, shape: ctx.enter_context(nc.psum_tensor(nm, shape, _F32))
        sem = lambda nm: ctx.enter_context(nc.semaphore(nm))
        t = sb("t_d", [DP, DW], _BF)
        h1 = sb("t_h1", [HD, T], _BF)
        h2 = sb("t_h2", [HD, T], _BF)
        warm = sb("t_warm", [HD, 1], _F32)
        xt_sb = sb("t_xt", [FD, T], _F32)
        p1 = ps("t_p1", [HD, T])
        p2 = ps("t_p2", [HD, T])
        pf = ps("t_pf", [T, FD])
        px = ps("t_px", [FD, T])
        sem_d1 = sem("sem_d1")
        sem_d2 = sem("sem_d2")
        pe_sem = sem("sem_pe")
        act_sem = sem("sem_act")
        dve_sem = sem("sem_dve")
        out_sem = sem("sem_out")

        st = t[0:IN_DIM, _C_ST:_C_ST + T]
        w1 = t[0:IN_DIM, _C_W1:_C_W1 + HD]
        w2 = t[0:HD, _C_W2:_C_W2 + HD]
        w3 = t[0:HD, _C_W3:_C_W3 + FD]
        b1 = t[0:HD, _C_B1:_C_B1 + 2].bitcast(_F32)
        b2 = t[0:HD, _C_B2:_C_B2 + 2].bitcast(_F32)
        ata = t[0:DP, _C_ATA:_C_ATA + T]
        fab = t[0:DP, _C_FAB:_C_FAB + FD]
        fabv = t[0:T, _C_FAB:_C_FAB + FD]   # per-sweep DVE write region

        block = ctx.enter_context(_LeanBlock(nc, 'blk'))

        # semaphore plan (DMA sems inc by 16 at transfer completion):
        #   sem_d1: DMA1 (state+weights+biases) = 16
        #   sem_d2: DMA2 (quadrature + fab consts) = 16
        #   pe_sem:  4 matmuls/sweep -> 4j+k after k-th matmul of sweep j
        #   act_sem: 2 tanhs/sweep   -> 2j+k
        #   dve_sem: 2 copies/sweep (fab, state), no state copy last sweep
        #   out_sem: output DMA (PSUM px -> HBM) = 16

        @block.gpsimd
        def _(gpsimd):
            nc.gpsimd.dma_start(t[0:DP, _C1:DW], d[0:DP, _C1:DW]).then_inc(sem_d2, 16)

        @block.sync
        def _(sync):
            sync.dma_start(t[0:HD, 0:_C1], d[0:HD, 0:_C1]).then_inc(sem_d1, 16)
            sync.wait_ge(dve_sem, 2 * nsweep)
            sync.dma_start(xt[:, :], xt_sb[:, :]).then_inc(out_sem, 16)
            sync.wait_ge(out_sem, 16)

        @block.tensor
        def _(tensor):
            tensor.wait_ge(sem_d1, 16)
            for j in range(nsweep):
                if j > 0:
                    tensor.wait_ge(dve_sem, 2 * j)
                nc.tensor.matmul(p1[:, :], w1, st, start=True, stop=True).then_inc(pe_sem, 1)
                tensor.wait_ge(act_sem, 2 * j + 1)
                nc.tensor.matmul(p2[:, :], w2, h1[:, :], start=True, stop=True).then_inc(pe_sem, 1)
                tensor.wait_ge(act_sem, 2 * j + 2)
                nc.tensor.matmul(pf[:, :], h2[:, :], w3, start=True, stop=True).then_inc(pe_sem, 1)
                tensor.wait_ge(dve_sem, 2 * j + 1)
                if j == 0:
                    tensor.wait_ge(sem_d2, 16)
                nc.tensor.matmul(px[:, :], fab, ata, start=True, stop=True).then_inc(pe_sem, 1)

        @block.scalar
        def _(scalar):
            # dep-free warm-up: zero a scratch tile, tanh it -> loads the
            # Tanh table while the input DMAs are still in flight.
            nc.scalar.mul(warm[:, :], warm[:, :], 0.0)
            nc.scalar.activation(warm[:, :], warm[:, :], tanh)
            for j in range(nsweep):
                scalar.wait_ge(pe_sem, 4 * j + 1)
                nc.scalar.activation(h1[:, :], p1[:, :], tanh, bias=b1).then_inc(act_sem, 1)
                scalar.wait_ge(pe_sem, 4 * j + 2)
                nc.scalar.activation(h2[:, :], p2[:, :], tanh, bias=b2).then_inc(act_sem, 1)

        @block.vector
        def _(vector):
            for j in range(nsweep):
                vector.wait_ge(pe_sem, 4 * j + 3)
                nc.vector.tensor_copy(fabv, pf[:, :]).then_inc(dve_sem, 1)
                vector.wait_ge(pe_sem, 4 * j + 4)
                if j < nsweep - 1:
                    nc.vector.tensor_copy(st[0:FD, :], px[:, :]).then_inc(dve_sem, 1)
                else:
                    nc.vector.tensor_copy(xt_sb[:, :], px[:, :]).then_inc(dve_sem, 1)

    return nc


_NC_CACHE = {}


def _get_nc(nsweep=NSWEEP):
    nsweep = int(nsweep)
    if nsweep not in _NC_CACHE:
        _NC_CACHE[nsweep] = _build_nc(nsweep)
    return _NC_CACHE[nsweep]


def _host_prep(x0, treatments, ts, W1, b1, W2, b2, W3, b3):
    ts64 = ts.astype(np.float64)
    x032 = np.ascontiguousarray(x0.reshape(FD), dtype=np.float32)

    # cumulative trapezoid quadrature matrix A [T,T]:
    # (A @ F)[t] ~= \int_{t_0}^{t_t} f dt  for F sampled at the grid times.
    h = np.diff(ts64)
    A = np.zeros((T, T), np.float64)
    for k in range(T - 1):
        A[k + 1] = A[k]
        A[k + 1, k] += h[k] / 2
        A[k + 1, k + 1] += h[k] / 2

    D = np.zeros((DP, DW), dtype=ml_dtypes.bfloat16)
    D[0:FD, _C_ST:_C_ST + T] = x032[:, None]
    D[FD:IN_DIM, _C_ST:_C_ST + T] = treatments.T
    D[0:IN_DIM, _C_W1:_C_W1 + HD] = W1
    D[0:HD, _C_W2:_C_W2 + HD] = W2
    D[0:HD, _C_W3:_C_W3 + FD] = W3
    D[0:T, _C_ATA:_C_ATA + T] = A.T
    D[T, _C_ATA:_C_ATA + T] = 1.0
    D[T + 1, _C_ATA:_C_ATA + T] = A.sum(axis=1)
    D[T, _C_FAB:_C_FAB + FD] = x032
    D[T + 1, _C_FAB:_C_FAB + FD] = b3
    # biases stay exact fp32, stored as raw bf16-bit pairs
    u16 = D.view(np.uint16)
    u16[0:HD, _C_B1:_C_B1 + 2] = np.ascontiguousarray(
        b1.reshape(HD, 1), dtype=np.float32).view(np.uint16)
    u16[0:HD, _C_B2:_C_B2 + 2] = np.ascontiguousarray(
        b2.reshape(HD, 1), dtype=np.float32).view(np.uint16)
    return {"d": D}


def kernel(x0, treatments, ts, W1, b1, W2, b2, W3, b3, _results=None, _plan=NSWEEP):
    in_map = _host_prep(x0, treatments, ts, W1, b1, W2, b2, W3, b3)
    nc = _get_nc(_plan)
    res = bass_utils.run_bass_kernel_spmd(
        nc, [in_map] * N_CORES, core_ids=list(range(N_CORES))
    )
    if _results is not None:
        _results.append(res)
    xt = res.results[0]["xt"]  # [FD, T]
    out = xt.T.reshape(T, 1, FD)
    return np.ascontiguousarray(out, dtype=np.float32)
